# revision 11
# baseline (speedup 1.0000x reference)
"""Tensor-parallel multi-head attention (RoPE) kernel for 8 Trainium2 cores.

Shapes (hardcoded): x [2, 2048, 1024], 16 heads x head_dim 64.
Sharding: core c -> batch b = c//4, head-group hg = c%4 (4 heads = 256
projection columns). Each core computes q/k/v projections for its head
columns, RoPE, attention, and a partial out-projection over its 256 rows
of o_w; the host sums the 4 partials per batch and adds o_b (plus the
v_b @ o_w term, which passes through attention linearly).

Device-side layout choices:
  - all matmul operands are bf16 (fp32 PSUM accumulation): measured end
    to end rel err ~5e-3 vs the 2e-2 gate.  bf16 halves the x/weight DMA
    and SBUF footprint, enables fast weight load (FWL) so LDWEIGHTS
    hides under the matmuls, and doubles DVE throughput for RoPE.
  - qT/kT stored transposed [head_dim on partitions, tokens on free]
    so QK^T contracts over partitions directly.
  - scores computed transposed S^T[k, q]; softmax max-subtraction is
    skipped (scores are O(+-6), exp is computed in fp32 from PSUM).
  - PV uses stationary [1 | V] so one accumulation produces the softmax
    denominators (partition 0 - partition_broadcast can only read
    partition 0 of a tile) and the unnormalized output (partitions 1-64).
  - softmax exp on ScalarE (ACT) is the pacing engine (~1147ns per
    [128,1024] tile, 147us total).  The attention loop is software-
    pipelined around it: per iteration the PE issues QK(tk) then
    PV(tk-1), so it never waits on the exp of the tile it just produced.
  - per head the q range is processed in two 1024-wide halves so the
    score scratch (2x [128,1024] = 4 banks) plus the PV accumulator
    ([65,1024] x 2 bufs = 4 banks) exactly fill PSUM, and each half's
    softmax normalization overlaps the next half's compute.
  - normalization: partition_broadcast the denominator row (GpSimd),
    reciprocal_approx_fast on 64 full lanes (DVE), one multiply.
"""

import sys
import numpy as np
import ml_dtypes

for p in ("/opt/trn_rl_repo", "/root/.axon_site/_ro/trn_rl_repo"):
    if p not in sys.path:
        sys.path.insert(0, p)

BF16NP = ml_dtypes.bfloat16

B, L, D = 2, 2048, 1024
H, HD = 16, 64
NCORES = 8
HG = 4                  # head-groups == cores per batch
EL = D // HG            # 256 projection columns per core
ET = EL // 128          # 2 e-tiles
DT = D // 128           # 8 d-tiles
TT = L // 128           # 16 token tiles
NH = H // HG            # 4 heads per core

_cache = {}

def _build():
    import concourse.mybir as mybir
    from concourse import bacc, tile

    F32 = mybir.dt.float32
    BF16 = mybir.dt.bfloat16
    AF = mybir.ActivationFunctionType

    nc = bacc.Bacc("TRN2", target_bir_lowering=False, debug=False,
                   num_devices=NCORES)

    xT = nc.dram_tensor("xT", [D, L], BF16, kind="ExternalInput").ap()
    wq = nc.dram_tensor("wq", [D, EL], BF16, kind="ExternalInput").ap()
    wk = nc.dram_tensor("wk", [D, EL], BF16, kind="ExternalInput").ap()
    wv = nc.dram_tensor("wv", [D, EL], BF16, kind="ExternalInput").ap()
    wo = nc.dram_tensor("wo", [EL, D], BF16, kind="ExternalInput").ap()
    bq = nc.dram_tensor("bq", [ET, 128, 1], F32, kind="ExternalInput").ap()
    bk = nc.dram_tensor("bk", [ET, 128, 1], F32, kind="ExternalInput").ap()
    cosb = nc.dram_tensor("cosb", [128, L], BF16, kind="ExternalInput").ap()
    onesd = nc.dram_tensor("onesd", [128, NH, 1], BF16, kind="ExternalInput").ap()
    sinb = nc.dram_tensor("sinb", [128, L], BF16, kind="ExternalInput").ap()
    outT = nc.dram_tensor("outT", [D, L], BF16, kind="ExternalOutput").ap()

    with tile.TileContext(nc) as tc:
        with tc.tile_pool(name="persist", bufs=1) as P:
            qT = [P.tile([128, L], BF16, name=f"qT{e}") for e in range(ET)]
            kT = [P.tile([128, L], BF16, name=f"kT{e}") for e in range(ET)]
            Vsb = [P.tile([128, NH * 65], BF16, name=f"V{t}") for t in range(TT)]
            ao = [P.tile([128, L], BF16, name=f"ao{e}") for e in range(ET)]
            wos = [P.tile([128, D], BF16, name=f"wo{e}") for e in range(ET)]
            # out-proj weights prefetched on the scalar DMA queue (idle
            # until attention) while sync carries the x stream.
            for e in range(ET):
                nc.scalar.dma_start(wos[e][:], wo[e * 128:(e + 1) * 128, :])

            # ---------------- Phase B: q/k/v projections + RoPE ----------
            with (
                tc.tile_pool(name="xw", bufs=1) as XW,
                tc.tile_pool(name="pb", bufs=2, space="PSUM") as PB,
                tc.tile_pool(name="ropet", bufs=2) as RT,
            ):
                # small tensors first so RoPE tables / biases are resident
                # long before the x stream finishes.  Inputs are split
                # across the two hardware DMA queues (sync + scalar): a
                # single queue measured only ~170GB/s, gating phase B.
                cosbt = XW.tile([128, L], BF16)
                sinbt = XW.tile([128, L], BF16)
                nc.scalar.dma_start(cosbt[:], cosb[:])
                nc.scalar.dma_start(sinbt[:], sinb[:])
                bqt = [XW.tile([128, 1], F32, name=f"bq{e}") for e in range(ET)]
                bkt = [XW.tile([128, 1], F32, name=f"bk{e}") for e in range(ET)]
                for e in range(ET):
                    nc.scalar.dma_start(bqt[e][:], bq[e])
                    nc.scalar.dma_start(bkt[e][:], bk[e])
                xts = [XW.tile([128, L], BF16, name=f"x{d}") for d in range(DT)]
                wqs = [XW.tile([128, EL], BF16, name=f"wq{d}") for d in range(DT)]
                wks = [XW.tile([128, EL], BF16, name=f"wk{d}") for d in range(DT)]
                wvs = [XW.tile([128, EL], BF16, name=f"wv{d}") for d in range(DT)]
                for d in range(DT):
                    nc.sync.dma_start(wqs[d][:], wq[d * 128:(d + 1) * 128, :])
                    # x halves ride both queues so each d-tile completes in
                    # half the single-queue time.
                    nc.sync.dma_start(xts[d][:, 0:1024],
                                      xT[d * 128:(d + 1) * 128, 0:1024])
                    nc.scalar.dma_start(xts[d][:, 1024:2048],
                                        xT[d * 128:(d + 1) * 128, 1024:2048])
                    nc.sync.dma_start(wks[d][:], wk[d * 128:(d + 1) * 128, :])
                    nc.scalar.dma_start(wvs[d][:], wv[d * 128:(d + 1) * 128, :])

                # e-tile 0 of q AND k first: head 0's attention only needs
                # those, so the exp pipeline starts as early as possible.
                for e in range(ET):
                    for wts, bts, dst in ((wqs, bqt, qT), (wks, bkt, kT)):
                        ps = PB.tile([128, L], F32, tag="ps")
                        for d in range(DT):
                            for c in range(0, L, 512):
                                nc.tensor.matmul(
                                    ps[:, c:c + 512],
                                    wts[d][:, e * 128:(e + 1) * 128],
                                    xts[d][:, c:c + 512],
                                    start=(d == 0), stop=(d == DT - 1),
                                    skip_group_check=True)
                        nc.vector.tensor_scalar_add(dst[e][:], ps[:], bts[e][:])
                        # RoPE: build rotate_half source via partition-shifted
                        # SBUF->SBUF DMA, then 2 muls + add (all bf16).
                        rs = RT.tile([128, L], BF16, tag="rs")
                        tmp = RT.tile([128, L], BF16, tag="tmp")
                        for g in range(4):
                            s0 = g * 32
                            d0 = s0 + 32 if g % 2 == 0 else s0 - 32
                            nc.sync.dma_start(rs[s0:s0 + 32, :],
                                              dst[e][d0:d0 + 32, :])
                        nc.vector.tensor_mul(tmp[:], dst[e][:], cosbt[:])
                        nc.vector.tensor_mul(rs[:], rs[:], sinbt[:])
                        nc.vector.tensor_add(dst[e][:], tmp[:], rs[:])

                for t in range(TT):
                    ps = PB.tile([128, EL], F32, tag="ps")
                    for d in range(DT):
                        nc.tensor.matmul(
                            ps[:], xts[d][:, t * 128:(t + 1) * 128],
                            wvs[d][:],
                            start=(d == 0), stop=(d == DT - 1),
                            skip_group_check=True)
                    dv = Vsb[t][:].rearrange("p (h c) -> p h c", c=65)
                    nc.vector.tensor_copy(
                        dv[:, :, 0:64],
                        ps[:].rearrange("p (h c) -> p h c", c=64))
                    nc.sync.dma_start(dv[:, :, 64:65], onesd[:])

            # ---------------- Phase C: attention per head ----------------
            # Software-pipelined: per iteration the PE runs QK(tk) (4 MMs
            # sharing one kh LDWEIGHTS) then PV(tk-1) (4 MMs sharing one
            # [V|1] LDWEIGHTS); the ACT exps of tile tk overlap the next
            # QK, so the slow engine (ACT, ~2294ns/tile) paces a gapless
            # pipeline.  Keeping 4 matmuls per LDWEIGHTS matters: a
            # 2-MM-per-LDW stream leaves enough PE micro-holes that the
            # HAM clock gate sticks at K=4/8 (measured 270us at 1.2GHz).
            with (
                tc.tile_pool(name="pscr", bufs=2, space="PSUM") as PS2,
                tc.tile_pool(name="pop", bufs=1, space="PSUM") as PO,
                tc.tile_pool(name="esb", bufs=4) as EP,
                tc.tile_pool(name="nsb", bufs=2) as SS,
            ):
                for h in range(NH):
                    e, off = divmod(h, 2)
                    off *= 64
                    qh = qT[e][off:off + 64, :]
                    kh = kT[e][off:off + 64, :]
                    oraw = SS.tile([65, L], F32, tag="oraw")
                    op = PO.tile([65, L], F32, tag="op")
                    prev = None
                    for tk in range(TT):
                        ebs = []
                        for c0 in (0, 1024):
                            sp = PS2.tile([128, 1024], F32, tag="scr")
                            for c in (0, 512):
                                nc.tensor.matmul(
                                    sp[:, c:c + 512],
                                    kh[:, tk * 128:(tk + 1) * 128],
                                    qh[:, c0 + c:c0 + c + 512],
                                    start=True, stop=True,
                                    skip_group_check=True)
                            eb = EP.tile([128, 1024], BF16, tag="eb")
                            nc.scalar.activation(eb[:], sp[:], AF.Exp,
                                                 scale=0.125)
                            ebs.append(eb)
                        if prev is not None:
                            pebs, ptk = prev
                            if h == 0 and ptk == 0:
                                # warm-keepers: the pipeline-fill wait on
                                # exp(0) is the one PE idle long enough to
                                # re-throttle the HAM clock gate (costing
                                # ~60us of half-clock attention).  Burn the
                                # wait with dummy matmuls into op regions
                                # that PV(0)'s start=True clears anyway.
                                for dc_ in range(8):
                                    nc.tensor.matmul(
                                        op[:, (dc_ % 4) * 512:
                                           (dc_ % 4) * 512 + 512],
                                        Vsb[0][:, h * 65:h * 65 + 65],
                                        qT[0][:, 0:512],
                                        start=True, stop=True,
                                        skip_group_check=True)
                            for i, c0 in enumerate((0, 1024)):
                                for c in (0, 512):
                                    nc.tensor.matmul(
                                        op[:, c0 + c:c0 + c + 512],
                                        Vsb[ptk][:, h * 65:h * 65 + 65],
                                        pebs[i][:, c:c + 512],
                                        start=(ptk == 0), stop=False,
                                        skip_group_check=True)
                        prev = (ebs, tk)
                    pebs, ptk = prev
                    for i, c0 in enumerate((0, 1024)):
                        for c in (0, 512):
                            nc.tensor.matmul(
                                op[:, c0 + c:c0 + c + 512],
                                Vsb[ptk][:, h * 65:h * 65 + 65],
                                pebs[i][:, c:c + 512],
                                start=False, stop=True,
                                skip_group_check=True)
                    # Drain op to SBUF immediately (both halves) so the
                    # single PSUM accumulator frees for the next head, then
                    # normalize per 1024-half so the last head's tail
                    # overlaps phase D's first chunks.  Denominators sit on
                    # op partition 64 ([V|1] stationary); partition_broadcast
                    # only reads partition 0 of a tile, so shift the row
                    # down with a tiny SBUF->SBUF DMA first.
                    for q0 in (0, 1024):
                        nc.vector.tensor_copy(oraw[:, q0:q0 + 1024],
                                              op[:, q0:q0 + 1024])
                    for q0 in (0, 1024):
                        dn = SS.tile([1, 1024], F32, tag="dn")
                        nc.sync.dma_start(dn[:], oraw[64:65, q0:q0 + 1024])
                        rbB = SS.tile([64, 1024], F32, tag="rbB")
                        nc.gpsimd.partition_broadcast(rbB[:], dn[:],
                                                      channels=64)
                        rbR = SS.tile([64, 1024], F32, tag="rbR")
                        nc.vector.reciprocal_approx_fast(rbR[:], rbB[:])
                        nc.vector.tensor_mul(
                            ao[e][off:off + 64, q0:q0 + 1024],
                            oraw[0:64, q0:q0 + 1024], rbR[:])

            # ---------------- Phase D: partial out-projection ------------
            # dc-outer so each wo stationary covers 4 matmuls (LDWEIGHTS
            # density matters for the HAM clock gate), drains alternate
            # ACT/DVE, output rides both DMA queues in bf16.
            with (
                tc.tile_pool(name="od", bufs=3) as OD,
                tc.tile_pool(name="pd", bufs=2, space="PSUM") as PD,
            ):
                for dc in range(DT):
                    pdt = PD.tile([128, L], F32, tag="pd")
                    if dc == 0:
                        # warm-keepers: bridge the ~8us normalize chain of
                        # the last head (DVE/GpSimd work) that would
                        # otherwise leave the PE idle and re-throttle the
                        # clock right before these 16us of matmuls.  The
                        # first real matmul's start=True clears the banks.
                        for dm in range(36):
                            nc.tensor.matmul(
                                pdt[:, (dm % 4) * 512:(dm % 4) * 512 + 512],
                                wos[0][:, 0:128],
                                qT[0][:, 0:512],
                                start=True, stop=True,
                                skip_group_check=True)
                    for e in range(ET):
                        for c in range(0, L, 512):
                            nc.tensor.matmul(
                                pdt[:, c:c + 512],
                                wos[e][:, dc * 128:(dc + 1) * 128],
                                ao[e][:, c:c + 512],
                                start=(e == 0), stop=(e == ET - 1),
                                skip_group_check=True)
                    osb = OD.tile([128, L], BF16, tag="osb")
                    if dc % 2 == 0:
                        nc.scalar.activation(osb[:], pdt[:], AF.Identity)
                    else:
                        nc.vector.tensor_copy(osb[:], pdt[:])
                    dma = nc.sync if dc % 2 == 0 else nc.scalar
                    dma.dma_start(outT[dc * 128:(dc + 1) * 128, :], osb[:])

    nc.compile()
    return nc


def _rope_tables():
    inv = 1.0 / (10000.0 ** (np.arange(0, HD, 2, dtype=np.float32) / HD))
    t = np.arange(L, dtype=np.float32)
    fr = t[:, None] * inv[None, :]                    # [L, 32]
    emb = np.concatenate([fr, fr], axis=1)            # [L, 64]
    cos, sin = np.cos(emb), np.sin(emb)               # [L, 64]
    # device layout [128, L]: row p covers head-dim i = p % 64, two heads
    # stacked per 128-partition tile; sin carries the rotate_half sign.
    i = np.arange(128) % HD
    cosb = cos.T[i, :]                                # [128, L]
    sg = np.where(i < HD // 2, -1.0, 1.0).astype(np.float32)
    sinb = sin.T[i, :] * sg[:, None]
    return np.ascontiguousarray(cosb).astype(BF16NP), \
        np.ascontiguousarray(sinb).astype(BF16NP)


def _in_maps(x, q_w, q_b, k_w, k_b, v_w, o_w):
    cosb, sinb = _rope_tables()
    qwT = np.asarray(q_w, np.float32).T.astype(BF16NP)  # [D, D] eff
    kwT = np.asarray(k_w, np.float32).T.astype(BF16NP)
    vwT = np.asarray(v_w, np.float32).T.astype(BF16NP)
    owT = np.asarray(o_w, np.float32).T.astype(BF16NP)
    xTb = [np.ascontiguousarray(x[b].T).astype(BF16NP) for b in range(B)]
    maps = []
    for c in range(NCORES):
        b, hg = divmod(c, HG)
        er = slice(hg * EL, (hg + 1) * EL)
        maps.append({
            "xT": xTb[b],
            "wq": np.ascontiguousarray(qwT[:, er]),
            "wk": np.ascontiguousarray(kwT[:, er]),
            "wv": np.ascontiguousarray(vwT[:, er]),
            "wo": np.ascontiguousarray(owT[er, :]),
            "bq": np.ascontiguousarray(
                np.asarray(q_b, np.float32)[er].reshape(ET, 128, 1)),
            "bk": np.ascontiguousarray(
                np.asarray(k_b, np.float32)[er].reshape(ET, 128, 1)),
            "cosb": cosb,
            "sinb": sinb,
            "onesd": np.ones((128, NH, 1), BF16NP),
        })
    return maps


def kernel(x, q_w, q_b, k_w, k_b, v_w, v_b, o_w, o_b):
    from concourse.bass_utils import run_bass_kernel_spmd

    x = np.asarray(x, np.float32)
    assert x.shape == (B, L, D), x.shape

    if "nc" not in _cache:
        _cache["nc"] = _build()
    nc = _cache["nc"]

    in_maps = _in_maps(x, q_w, q_b, k_w, k_b, v_w, o_w)
    res = run_bass_kernel_spmd(nc, in_maps, list(range(NCORES)))

    out = np.zeros((B, L, D), np.float32)
    for c in range(NCORES):
        b = c // HG
        out[b] += res.results[c]["outT"].T.astype(np.float32)
    # o_b, plus v_b's contribution (v_b flows through softmax-weighted
    # averaging unchanged, then through the out-projection).
    extra = np.asarray(o_b, np.float32) + \
        np.asarray(v_b, np.float32) @ np.asarray(o_w, np.float32).T
    out += extra[None, None, :]
    return out


# revision 15
# speedup vs baseline: 1.0397x; 1.0397x over previous
"""Tensor-parallel multi-head attention (RoPE) kernel for 8 Trainium2 cores.

Shapes (hardcoded): x [2, 2048, 1024], 16 heads x head_dim 64.
Sharding: core c -> batch b = c//4, head-group hg = c%4 (4 heads = 256
projection columns). Each core computes q/k/v projections for its head
columns, RoPE, attention, and a partial out-projection over its 256 rows
of o_w; the host sums the 4 partials per batch and adds o_b (plus the
v_b @ o_w term, which passes through attention linearly).

Device-side layout choices:
  - all matmul operands are bf16 (fp32 PSUM accumulation): measured end
    to end rel err ~5e-3 vs the 2e-2 gate.  bf16 halves the x/weight DMA
    and SBUF footprint, enables fast weight load (FWL) so LDWEIGHTS
    hides under the matmuls, and doubles DVE throughput for RoPE.
  - qT/kT stored transposed [head_dim on partitions, tokens on free]
    so QK^T contracts over partitions directly.
  - scores computed transposed S^T[k, q]; softmax max-subtraction is
    skipped (scores are O(+-6), exp is computed in fp32 from PSUM).
  - PV uses stationary [1 | V] so one accumulation produces the softmax
    denominators (partition 0 - partition_broadcast can only read
    partition 0 of a tile) and the unnormalized output (partitions 1-64).
  - softmax exp on ScalarE (ACT) is the pacing engine (~1147ns per
    [128,1024] tile, 147us total).  The attention loop is software-
    pipelined around it: per iteration the PE issues QK(tk) then
    PV(tk-1), so it never waits on the exp of the tile it just produced.
  - per head the q range is processed in two 1024-wide halves so the
    score scratch (2x [128,1024] = 4 banks) plus the PV accumulator
    ([65,1024] x 2 bufs = 4 banks) exactly fill PSUM, and each half's
    softmax normalization overlaps the next half's compute.
  - normalization: partition_broadcast the denominator row (GpSimd),
    reciprocal_approx_fast on 64 full lanes (DVE), one multiply.
"""

import sys
import numpy as np
import ml_dtypes

for p in ("/opt/trn_rl_repo", "/root/.axon_site/_ro/trn_rl_repo"):
    if p not in sys.path:
        sys.path.insert(0, p)

BF16NP = ml_dtypes.bfloat16

B, L, D = 2, 2048, 1024
H, HD = 16, 64
NCORES = 8
HG = 4                  # head-groups == cores per batch
EL = D // HG            # 256 projection columns per core
ET = EL // 128          # 2 e-tiles
DT = D // 128           # 8 d-tiles
TT = L // 128           # 16 token tiles
NH = H // HG            # 4 heads per core

_cache = {}

def _build():
    import concourse.mybir as mybir
    from concourse import bacc, tile

    F32 = mybir.dt.float32
    BF16 = mybir.dt.bfloat16
    AF = mybir.ActivationFunctionType

    nc = bacc.Bacc("TRN2", target_bir_lowering=False, debug=False,
                   num_devices=NCORES)

    xT = nc.dram_tensor("xT", [D, L], BF16, kind="ExternalInput").ap()
    wq = nc.dram_tensor("wq", [D, EL], BF16, kind="ExternalInput").ap()
    wk = nc.dram_tensor("wk", [D, EL], BF16, kind="ExternalInput").ap()
    wv = nc.dram_tensor("wv", [D, EL], BF16, kind="ExternalInput").ap()
    wo = nc.dram_tensor("wo", [EL, D], BF16, kind="ExternalInput").ap()
    bq = nc.dram_tensor("bq", [ET, 128, 1], F32, kind="ExternalInput").ap()
    bk = nc.dram_tensor("bk", [ET, 128, 1], F32, kind="ExternalInput").ap()
    cosb = nc.dram_tensor("cosb", [128, L], BF16, kind="ExternalInput").ap()
    onesd = nc.dram_tensor("onesd", [128, NH, 1], BF16, kind="ExternalInput").ap()
    sinb = nc.dram_tensor("sinb", [128, L], BF16, kind="ExternalInput").ap()
    outT = nc.dram_tensor("outT", [D, L], BF16, kind="ExternalOutput").ap()

    with tile.TileContext(nc) as tc:
        with tc.tile_pool(name="persist", bufs=1) as P:
            qT = [P.tile([128, L], BF16, name=f"qT{e}") for e in range(ET)]
            kT = [P.tile([128, L], BF16, name=f"kT{e}") for e in range(ET)]
            Vsb = [P.tile([128, NH * 65], BF16, name=f"V{t}") for t in range(TT)]
            ao = [P.tile([128, L], BF16, name=f"ao{e}") for e in range(ET)]
            wos = [P.tile([128, D], BF16, name=f"wo{e}") for e in range(ET)]

            # ---------------- Phase B: q/k/v projections + RoPE ----------
            with (
                tc.tile_pool(name="xw", bufs=1) as XW,
                tc.tile_pool(name="pb", bufs=2, space="PSUM") as PB,
                tc.tile_pool(name="ropet", bufs=2) as RT,
            ):
                # small tensors first so RoPE tables / biases are resident
                # long before the x stream finishes.  Inputs are split
                # across the two hardware DMA queues (sync + scalar): a
                # single queue measured only ~170GB/s, gating phase B.
                cosbt = XW.tile([128, L], BF16)
                sinbt = XW.tile([128, L], BF16)
                nc.scalar.dma_start(cosbt[:], cosb[:])
                nc.scalar.dma_start(sinbt[:], sinb[:])
                bqt = [XW.tile([128, 1], F32, name=f"bq{e}") for e in range(ET)]
                bkt = [XW.tile([128, 1], F32, name=f"bk{e}") for e in range(ET)]
                for e in range(ET):
                    nc.scalar.dma_start(bqt[e][:], bq[e])
                    nc.scalar.dma_start(bkt[e][:], bk[e])
                xts = [XW.tile([128, L], BF16, name=f"x{d}") for d in range(DT)]
                wqs = [XW.tile([128, EL], BF16, name=f"wq{d}") for d in range(DT)]
                wks = [XW.tile([128, EL], BF16, name=f"wk{d}") for d in range(DT)]
                wvs = [XW.tile([128, EL], BF16, name=f"wv{d}") for d in range(DT)]
                for d in range(DT):
                    nc.sync.dma_start(wqs[d][:], wq[d * 128:(d + 1) * 128, :])
                    # x halves ride both queues so each d-tile completes in
                    # half the single-queue time.
                    nc.sync.dma_start(xts[d][:, 0:1024],
                                      xT[d * 128:(d + 1) * 128, 0:1024])
                    nc.scalar.dma_start(xts[d][:, 1024:2048],
                                        xT[d * 128:(d + 1) * 128, 1024:2048])
                    nc.sync.dma_start(wks[d][:], wk[d * 128:(d + 1) * 128, :])
                    nc.scalar.dma_start(wvs[d][:], wv[d * 128:(d + 1) * 128, :])
                # out-proj weights last: only needed by phase D.
                for e in range(ET):
                    nc.sync.dma_start(wos[e][:], wo[e * 128:(e + 1) * 128, :])

                # e-tile 0 of q AND k first: head 0's attention only needs
                # those, so the exp pipeline starts as early as possible.
                # The PSUM->SBUF bias-add runs on ScalarE (idle here) so the
                # DVE only carries the three RoPE tensor ops - the serial
                # DVE queue was pacing all of phase B.
                for e in range(ET):
                    for wts, bts, dst in ((wqs, bqt, qT), (wks, bkt, kT)):
                        ps = PB.tile([128, L], F32, tag="ps")
                        for d in range(DT):
                            for c in range(0, L, 512):
                                nc.tensor.matmul(
                                    ps[:, c:c + 512],
                                    wts[d][:, e * 128:(e + 1) * 128],
                                    xts[d][:, c:c + 512],
                                    start=(d == 0), stop=(d == DT - 1),
                                    skip_group_check=True)
                        nc.scalar.activation(dst[e][:], ps[:], AF.Identity,
                                             bias=bts[e][:])
                        # RoPE: build rotate_half source via partition-shifted
                        # SBUF->SBUF DMA, then 2 muls + add (all bf16).
                        rs = RT.tile([128, L], BF16, tag="rs")
                        tmp = RT.tile([128, L], BF16, tag="tmp")
                        for g in range(4):
                            s0 = g * 32
                            d0 = s0 + 32 if g % 2 == 0 else s0 - 32
                            nc.sync.dma_start(rs[s0:s0 + 32, :],
                                              dst[e][d0:d0 + 32, :])
                        nc.vector.tensor_mul(tmp[:], dst[e][:], cosbt[:])
                        nc.vector.tensor_mul(rs[:], rs[:], sinbt[:])
                        nc.vector.tensor_add(dst[e][:], tmp[:], rs[:])

                for t in range(TT):
                    ps = PB.tile([128, EL], F32, tag="ps")
                    for d in range(DT):
                        nc.tensor.matmul(
                            ps[:], xts[d][:, t * 128:(t + 1) * 128],
                            wvs[d][:],
                            start=(d == 0), stop=(d == DT - 1),
                            skip_group_check=True)
                    dv = Vsb[t][:].rearrange("p (h c) -> p h c", c=65)
                    # V drain on ScalarE too: the DVE queue (RoPE) must not
                    # pace Vsb availability for head 0's PV stream.
                    nc.scalar.activation(
                        dv[:, :, 0:64],
                        ps[:].rearrange("p (h c) -> p h c", c=64),
                        AF.Identity)
                    nc.sync.dma_start(dv[:, :, 64:65], onesd[:])

            # ---------------- Phase C: attention per head ----------------
            # Software-pipelined: per iteration the PE runs QK(tk) (4 MMs
            # sharing one kh LDWEIGHTS) then PV(tk-1) (4 MMs sharing one
            # [V|1] LDWEIGHTS); the ACT exps of tile tk overlap the next
            # QK, so the slow engine (ACT, ~2294ns/tile) paces a gapless
            # pipeline.  Keeping 4 matmuls per LDWEIGHTS matters: a
            # 2-MM-per-LDW stream leaves enough PE micro-holes that the
            # HAM clock gate sticks at K=4/8 (measured 270us at 1.2GHz).
            with (
                tc.tile_pool(name="pscr", bufs=2, space="PSUM") as PS2,
                tc.tile_pool(name="pop", bufs=1, space="PSUM") as PO,
                tc.tile_pool(name="esb", bufs=4) as EP,
                tc.tile_pool(name="nsb", bufs=2) as SS,
            ):
                # Flat (h, tk) stream with PV trailing by one iteration and
                # the per-head normalize emitted after the NEXT head's first
                # QK: the PE crosses head boundaries without draining the
                # pipeline, so the only idle long enough to trip the HAM
                # clock gate is the initial fill (bridged with dummies).
                def normalize(h):
                    e, off = divmod(h, 2)
                    off *= 64
                    oraw = oraws[h]
                    for q0 in (0, 1024):
                        nc.vector.tensor_copy(oraw[:, q0:q0 + 1024],
                                              ops[h][:, q0:q0 + 1024])
                    # Denominators sit on op partition 64 ([V|1]
                    # stationary); partition_broadcast only reads partition
                    # 0 of a tile, so shift the row down with a tiny
                    # SBUF->SBUF DMA first.
                    for q0 in (0, 1024):
                        dn = SS.tile([1, 1024], F32, tag="dn")
                        nc.sync.dma_start(dn[:], oraw[64:65, q0:q0 + 1024])
                        rbB = SS.tile([64, 1024], F32, tag="rbB")
                        nc.gpsimd.partition_broadcast(rbB[:], dn[:],
                                                      channels=64)
                        rbR = SS.tile([64, 1024], F32, tag="rbR")
                        nc.vector.reciprocal_approx_fast(rbR[:], rbB[:])
                        nc.vector.tensor_mul(
                            ao[e][off:off + 64, q0:q0 + 1024],
                            oraw[0:64, q0:q0 + 1024], rbR[:])

                ops = {}
                oraws = {}
                prev = None
                for it in range(NH * TT):
                    h, tk = divmod(it, TT)
                    e, off = divmod(h, 2)
                    off *= 64
                    qh = qT[e][off:off + 64, :]
                    kh = kT[e][off:off + 64, :]
                    if tk == 0:
                        ops[h] = PO.tile([65, L], F32, tag="op", name=f"op{h}")
                        oraws[h] = SS.tile([65, L], F32, tag="oraw",
                                           name=f"oraw{h}")
                    ebs = []
                    for c0 in (0, 1024):
                        sp = PS2.tile([128, 1024], F32, tag="scr")
                        for c in (0, 512):
                            nc.tensor.matmul(
                                sp[:, c:c + 512],
                                kh[:, tk * 128:(tk + 1) * 128],
                                qh[:, c0 + c:c0 + c + 512],
                                start=True, stop=True,
                                skip_group_check=True)
                        eb = EP.tile([128, 1024], BF16, tag="eb")
                        nc.scalar.activation(eb[:], sp[:], AF.Exp,
                                             scale=0.125)
                        ebs.append(eb)
                    if prev is not None:
                        pebs, ph, ptk = prev
                        if it == 1:
                            # warm-keepers: the pipeline-fill wait on exp(0)
                            # is a PE idle long enough to re-throttle the
                            # HAM clock gate (costing ~60us of half-clock
                            # attention).  Burn it with dummy matmuls into
                            # op regions that PV(0)'s start=True clears.
                            for dm in range(8):
                                nc.tensor.matmul(
                                    ops[0][:, (dm % 4) * 512:
                                           (dm % 4) * 512 + 512],
                                    Vsb[0][:, 0:65],
                                    qT[0][:, 0:512],
                                    start=True, stop=True,
                                    skip_group_check=True)
                        for i, c0 in enumerate((0, 1024)):
                            for c in (0, 512):
                                nc.tensor.matmul(
                                    ops[ph][:, c0 + c:c0 + c + 512],
                                    Vsb[ptk][:, ph * 65:ph * 65 + 65],
                                    pebs[i][:, c:c + 512],
                                    start=(ptk == 0), stop=(ptk == TT - 1),
                                    skip_group_check=True)
                        if ptk == TT - 1:
                            normalize(ph)
                    prev = (ebs, h, tk)
                pebs, ph, ptk = prev
                for i, c0 in enumerate((0, 1024)):
                    for c in (0, 512):
                        nc.tensor.matmul(
                            ops[ph][:, c0 + c:c0 + c + 512],
                            Vsb[ptk][:, ph * 65:ph * 65 + 65],
                            pebs[i][:, c:c + 512],
                            start=False, stop=True,
                            skip_group_check=True)
                normalize(ph)

            # ---------------- Phase D: partial out-projection ------------
            # dc-outer so each wo stationary covers 4 matmuls (LDWEIGHTS
            # density matters for the HAM clock gate), drains alternate
            # ACT/DVE, output rides both DMA queues in bf16.
            with (
                tc.tile_pool(name="od", bufs=3) as OD,
                tc.tile_pool(name="pd", bufs=2, space="PSUM") as PD,
            ):
                for dc in range(DT):
                    pdt = PD.tile([128, L], F32, tag="pd")
                    if dc == 0:
                        # warm-keepers: bridge the ~8us normalize chain of
                        # the last head (DVE/GpSimd work) that would
                        # otherwise leave the PE idle and re-throttle the
                        # clock right before these 16us of matmuls.  The
                        # first real matmul's start=True clears the banks.
                        for dm in range(36):
                            nc.tensor.matmul(
                                pdt[:, (dm % 4) * 512:(dm % 4) * 512 + 512],
                                wos[0][:, 0:128],
                                qT[0][:, 0:512],
                                start=True, stop=True,
                                skip_group_check=True)
                    for e in range(ET):
                        for c in range(0, L, 512):
                            nc.tensor.matmul(
                                pdt[:, c:c + 512],
                                wos[e][:, dc * 128:(dc + 1) * 128],
                                ao[e][:, c:c + 512],
                                start=(e == 0), stop=(e == ET - 1),
                                skip_group_check=True)
                    osb = OD.tile([128, L], BF16, tag="osb")
                    if dc % 2 == 0:
                        nc.scalar.activation(osb[:], pdt[:], AF.Identity)
                    else:
                        nc.vector.tensor_copy(osb[:], pdt[:])
                    dma = nc.sync if dc % 2 == 0 else nc.scalar
                    dma.dma_start(outT[dc * 128:(dc + 1) * 128, :], osb[:])

    nc.compile()
    return nc


def _rope_tables():
    inv = 1.0 / (10000.0 ** (np.arange(0, HD, 2, dtype=np.float32) / HD))
    t = np.arange(L, dtype=np.float32)
    fr = t[:, None] * inv[None, :]                    # [L, 32]
    emb = np.concatenate([fr, fr], axis=1)            # [L, 64]
    cos, sin = np.cos(emb), np.sin(emb)               # [L, 64]
    # device layout [128, L]: row p covers head-dim i = p % 64, two heads
    # stacked per 128-partition tile; sin carries the rotate_half sign.
    i = np.arange(128) % HD
    cosb = cos.T[i, :]                                # [128, L]
    sg = np.where(i < HD // 2, -1.0, 1.0).astype(np.float32)
    sinb = sin.T[i, :] * sg[:, None]
    return np.ascontiguousarray(cosb).astype(BF16NP), \
        np.ascontiguousarray(sinb).astype(BF16NP)


def _in_maps(x, q_w, q_b, k_w, k_b, v_w, o_w):
    cosb, sinb = _rope_tables()
    qwT = np.asarray(q_w, np.float32).T.astype(BF16NP)  # [D, D] eff
    kwT = np.asarray(k_w, np.float32).T.astype(BF16NP)
    vwT = np.asarray(v_w, np.float32).T.astype(BF16NP)
    owT = np.asarray(o_w, np.float32).T.astype(BF16NP)
    xTb = [np.ascontiguousarray(x[b].T).astype(BF16NP) for b in range(B)]
    maps = []
    for c in range(NCORES):
        b, hg = divmod(c, HG)
        er = slice(hg * EL, (hg + 1) * EL)
        maps.append({
            "xT": xTb[b],
            "wq": np.ascontiguousarray(qwT[:, er]),
            "wk": np.ascontiguousarray(kwT[:, er]),
            "wv": np.ascontiguousarray(vwT[:, er]),
            "wo": np.ascontiguousarray(owT[er, :]),
            "bq": np.ascontiguousarray(
                np.asarray(q_b, np.float32)[er].reshape(ET, 128, 1)),
            "bk": np.ascontiguousarray(
                np.asarray(k_b, np.float32)[er].reshape(ET, 128, 1)),
            "cosb": cosb,
            "sinb": sinb,
            "onesd": np.ones((128, NH, 1), BF16NP),
        })
    return maps


def kernel(x, q_w, q_b, k_w, k_b, v_w, v_b, o_w, o_b):
    from concourse.bass_utils import run_bass_kernel_spmd

    x = np.asarray(x, np.float32)
    assert x.shape == (B, L, D), x.shape

    if "nc" not in _cache:
        _cache["nc"] = _build()
    nc = _cache["nc"]

    in_maps = _in_maps(x, q_w, q_b, k_w, k_b, v_w, o_w)
    res = run_bass_kernel_spmd(nc, in_maps, list(range(NCORES)))

    out = np.zeros((B, L, D), np.float32)
    for c in range(NCORES):
        b = c // HG
        out[b] += res.results[c]["outT"].T.astype(np.float32)
    # o_b, plus v_b's contribution (v_b flows through softmax-weighted
    # averaging unchanged, then through the out-projection).
    extra = np.asarray(o_b, np.float32) + \
        np.asarray(v_b, np.float32) @ np.asarray(o_w, np.float32).T
    out += extra[None, None, :]
    return out


# revision 18
# speedup vs baseline: 1.0428x; 1.0030x over previous
"""Tensor-parallel multi-head attention (RoPE) kernel for 8 Trainium2 cores.

Shapes (hardcoded): x [2, 2048, 1024], 16 heads x head_dim 64.
Sharding: core c -> batch b = c//4, head-group hg = c%4 (4 heads = 256
projection columns). Each core computes q/k/v projections for its head
columns, RoPE, attention, and a partial out-projection over its 256 rows
of o_w; the host sums the 4 partials per batch and adds o_b (plus the
v_b @ o_w term, which passes through attention linearly).

Device-side layout choices:
  - all matmul operands are bf16 (fp32 PSUM accumulation): measured end
    to end rel err ~5e-3 vs the 2e-2 gate.  bf16 halves the x/weight DMA
    and SBUF footprint, enables fast weight load (FWL) so LDWEIGHTS
    hides under the matmuls, and doubles DVE throughput for RoPE.
  - qT/kT stored transposed [head_dim on partitions, tokens on free]
    so QK^T contracts over partitions directly.
  - scores computed transposed S^T[k, q]; softmax max-subtraction is
    skipped (scores are O(+-6), exp is computed in fp32 from PSUM).
  - PV uses stationary [1 | V] so one accumulation produces the softmax
    denominators (partition 0 - partition_broadcast can only read
    partition 0 of a tile) and the unnormalized output (partitions 1-64).
  - softmax exp on ScalarE (ACT) is the pacing engine (~1147ns per
    [128,1024] tile, 147us total).  The attention loop is software-
    pipelined around it: per iteration the PE issues QK(tk) then
    PV(tk-1), so it never waits on the exp of the tile it just produced.
  - per head the q range is processed in two 1024-wide halves so the
    score scratch (2x [128,1024] = 4 banks) plus the PV accumulator
    ([65,1024] x 2 bufs = 4 banks) exactly fill PSUM, and each half's
    softmax normalization overlaps the next half's compute.
  - normalization: partition_broadcast the denominator row (GpSimd),
    reciprocal_approx_fast on 64 full lanes (DVE), one multiply.
"""

import sys
import numpy as np
import ml_dtypes

for p in ("/opt/trn_rl_repo", "/root/.axon_site/_ro/trn_rl_repo"):
    if p not in sys.path:
        sys.path.insert(0, p)

BF16NP = ml_dtypes.bfloat16

B, L, D = 2, 2048, 1024
H, HD = 16, 64
NCORES = 8
HG = 4                  # head-groups == cores per batch
EL = D // HG            # 256 projection columns per core
ET = EL // 128          # 2 e-tiles
DT = D // 128           # 8 d-tiles
TT = L // 128           # 16 token tiles
NH = H // HG            # 4 heads per core

_cache = {}

def _build():
    import concourse.mybir as mybir
    from concourse import bacc, tile

    F32 = mybir.dt.float32
    BF16 = mybir.dt.bfloat16
    AF = mybir.ActivationFunctionType

    nc = bacc.Bacc("TRN2", target_bir_lowering=False, debug=False,
                   num_devices=NCORES)

    xT = nc.dram_tensor("xT", [D, L], BF16, kind="ExternalInput").ap()
    wq = nc.dram_tensor("wq", [D, EL], BF16, kind="ExternalInput").ap()
    wk = nc.dram_tensor("wk", [D, EL], BF16, kind="ExternalInput").ap()
    wv = nc.dram_tensor("wv", [D, EL], BF16, kind="ExternalInput").ap()
    wo = nc.dram_tensor("wo", [EL, D], BF16, kind="ExternalInput").ap()
    bq = nc.dram_tensor("bq", [ET, 128, 1], F32, kind="ExternalInput").ap()
    bk = nc.dram_tensor("bk", [ET, 128, 1], F32, kind="ExternalInput").ap()
    cosb = nc.dram_tensor("cosb", [128, L], BF16, kind="ExternalInput").ap()
    onesd = nc.dram_tensor("onesd", [128, NH, 1], BF16, kind="ExternalInput").ap()
    sinb = nc.dram_tensor("sinb", [128, L], BF16, kind="ExternalInput").ap()
    outT = nc.dram_tensor("outT", [D, L], BF16, kind="ExternalOutput").ap()

    with tile.TileContext(nc) as tc:
        with tc.tile_pool(name="persist", bufs=1) as P:
            qT = [P.tile([128, L], BF16, name=f"qT{e}") for e in range(ET)]
            kT = [P.tile([128, L], BF16, name=f"kT{e}") for e in range(ET)]
            Vsb = [P.tile([128, NH * 65], BF16, name=f"V{t}") for t in range(TT)]
            ao = [P.tile([128, L], BF16, name=f"ao{e}") for e in range(ET)]
            wos = [P.tile([128, D], BF16, name=f"wo{e}") for e in range(ET)]

            # ---------------- Phase B: q/k/v projections + RoPE ----------
            with (
                tc.tile_pool(name="xw", bufs=1) as XW,
                tc.tile_pool(name="pb", bufs=2, space="PSUM") as PB,
                tc.tile_pool(name="ropet", bufs=2) as RT,
            ):
                # small tensors first so RoPE tables / biases are resident
                # long before the x stream finishes.  Inputs are split
                # across the two hardware DMA queues (sync + scalar): a
                # single queue measured only ~170GB/s, gating phase B.
                cosbt = XW.tile([128, L], BF16)
                sinbt = XW.tile([128, L], BF16)
                nc.scalar.dma_start(cosbt[:], cosb[:])
                nc.scalar.dma_start(sinbt[:], sinb[:])
                bqt = [XW.tile([128, 1], F32, name=f"bq{e}") for e in range(ET)]
                bkt = [XW.tile([128, 1], F32, name=f"bk{e}") for e in range(ET)]
                for e in range(ET):
                    nc.scalar.dma_start(bqt[e][:], bq[e])
                    nc.scalar.dma_start(bkt[e][:], bk[e])
                xts = [XW.tile([128, L], BF16, name=f"x{d}") for d in range(DT)]
                wqs = [XW.tile([128, EL], BF16, name=f"wq{d}") for d in range(DT)]
                wks = [XW.tile([128, EL], BF16, name=f"wk{d}") for d in range(DT)]
                wvs = [XW.tile([128, EL], BF16, name=f"wv{d}") for d in range(DT)]
                for d in range(DT):
                    nc.sync.dma_start(wqs[d][:], wq[d * 128:(d + 1) * 128, :])
                    # x halves ride both queues so each d-tile completes in
                    # half the single-queue time.
                    nc.sync.dma_start(xts[d][:, 0:1024],
                                      xT[d * 128:(d + 1) * 128, 0:1024])
                    nc.scalar.dma_start(xts[d][:, 1024:2048],
                                        xT[d * 128:(d + 1) * 128, 1024:2048])
                    nc.sync.dma_start(wks[d][:], wk[d * 128:(d + 1) * 128, :])
                    nc.scalar.dma_start(wvs[d][:], wv[d * 128:(d + 1) * 128, :])
                # out-proj weights last: only needed by phase D.
                for e in range(ET):
                    nc.sync.dma_start(wos[e][:], wo[e * 128:(e + 1) * 128, :])

                # e-tile 0 of q AND k first: head 0's attention only needs
                # those, so the exp pipeline starts as early as possible.
                # The PSUM->SBUF bias-add runs on ScalarE (idle here) so the
                # DVE only carries the three RoPE tensor ops - the serial
                # DVE queue was pacing all of phase B.
                for e in range(ET):
                    for wts, bts, dst in ((wqs, bqt, qT), (wks, bkt, kT)):
                        # 1024-wide psum chunks ([128,1024] x 2 bufs = 4
                        # banks) leave room for the V-proj slots below; the
                        # 1.15us bias-Identity hides under the next chunk's
                        # 3.4us matmul group.
                        for lh in (0, 1024):
                            ps = PB.tile([128, 1024], F32, tag="ps")
                            for d in range(DT):
                                for c in (0, 512):
                                    nc.tensor.matmul(
                                        ps[:, c:c + 512],
                                        wts[d][:, e * 128:(e + 1) * 128],
                                        xts[d][:, lh + c:lh + c + 512],
                                        start=(d == 0), stop=(d == DT - 1),
                                        skip_group_check=True)
                            nc.scalar.activation(dst[e][:, lh:lh + 1024],
                                                 ps[:], AF.Identity,
                                                 bias=bts[e][:])
                        # RoPE: build rotate_half source via partition-shifted
                        # SBUF->SBUF DMA, then 2 muls + add (all bf16).
                        rs = RT.tile([128, L], BF16, tag="rs")
                        tmp = RT.tile([128, L], BF16, tag="tmp")
                        for g in range(4):
                            s0 = g * 32
                            d0 = s0 + 32 if g % 2 == 0 else s0 - 32
                            nc.sync.dma_start(rs[s0:s0 + 32, :],
                                              dst[e][d0:d0 + 32, :])
                        nc.vector.tensor_mul(tmp[:], dst[e][:], cosbt[:])
                        nc.vector.tensor_mul(rs[:], rs[:], sinbt[:])
                        nc.vector.tensor_add(dst[e][:], tmp[:], rs[:])

                for t in range(TT):
                    # small dedicated psum slots (bufs=4) so the V matmul
                    # stream is not drain-ping-pong paced; drains alternate
                    # ACT/DVE to dodge head-of-line blocking behind the
                    # 2us bias-Identities / RoPE ops queued on either.
                    ps = PB.tile([128, EL], F32, tag="vps", bufs=4)
                    for d in range(DT):
                        nc.tensor.matmul(
                            ps[:], xts[d][:, t * 128:(t + 1) * 128],
                            wvs[d][:],
                            start=(d == 0), stop=(d == DT - 1),
                            skip_group_check=True)
                    dv = Vsb[t][:].rearrange("p (h c) -> p h c", c=65)
                    psv = ps[:].rearrange("p (h c) -> p h c", c=64)
                    if t % 2 == 0:
                        nc.scalar.activation(dv[:, :, 0:64], psv, AF.Identity)
                    else:
                        nc.vector.tensor_copy(dv[:, :, 0:64], psv)
                    nc.sync.dma_start(dv[:, :, 64:65], onesd[:])

            # ---------------- Phase C: attention per head ----------------
            # Software-pipelined: per iteration the PE runs QK(tk) (4 MMs
            # sharing one kh LDWEIGHTS) then PV(tk-1) (4 MMs sharing one
            # [V|1] LDWEIGHTS); the ACT exps of tile tk overlap the next
            # QK, so the slow engine (ACT, ~2294ns/tile) paces a gapless
            # pipeline.  Keeping 4 matmuls per LDWEIGHTS matters: a
            # 2-MM-per-LDW stream leaves enough PE micro-holes that the
            # HAM clock gate sticks at K=4/8 (measured 270us at 1.2GHz).
            with (
                tc.tile_pool(name="pscr", bufs=2, space="PSUM") as PS2,
                tc.tile_pool(name="pop", bufs=1, space="PSUM") as PO,
                tc.tile_pool(name="esb", bufs=4) as EP,
                tc.tile_pool(name="nsb", bufs=2) as SS,
            ):
                # Flat (h, tk) stream with PV trailing by one iteration and
                # the per-head normalize emitted after the NEXT head's first
                # QK: the PE crosses head boundaries without draining the
                # pipeline, so the only idle long enough to trip the HAM
                # clock gate is the initial fill (bridged with dummies).
                def normalize(h):
                    e, off = divmod(h, 2)
                    off *= 64
                    oraw = oraws[h]
                    # drain in 512 chunks: subtile WAR tracking then lets the
                    # next head's first PV start as soon as its slice is
                    # drained, instead of idling the PE ~2us per head
                    # boundary (each such idle risks a HAM re-throttle that
                    # costs ~100us of half-clock attention).
                    for q0 in range(0, L, 512):
                        nc.vector.tensor_copy(oraw[:, q0:q0 + 512],
                                              ops[h][:, q0:q0 + 512])
                    # Denominators sit on op partition 64 ([V|1]
                    # stationary); partition_broadcast only reads partition
                    # 0 of a tile, so shift the row down with a tiny
                    # SBUF->SBUF DMA first.
                    for q0 in (0, 1024):
                        dn = SS.tile([1, 1024], F32, tag="dn")
                        nc.sync.dma_start(dn[:], oraw[64:65, q0:q0 + 1024])
                        rbB = SS.tile([64, 1024], F32, tag="rbB")
                        nc.gpsimd.partition_broadcast(rbB[:], dn[:],
                                                      channels=64)
                        rbR = SS.tile([64, 1024], F32, tag="rbR")
                        nc.vector.reciprocal_approx_fast(rbR[:], rbB[:])
                        nc.vector.tensor_mul(
                            ao[e][off:off + 64, q0:q0 + 1024],
                            oraw[0:64, q0:q0 + 1024], rbR[:])

                ops = {}
                oraws = {}
                prev = None
                for it in range(NH * TT):
                    h, tk = divmod(it, TT)
                    e, off = divmod(h, 2)
                    off *= 64
                    qh = qT[e][off:off + 64, :]
                    kh = kT[e][off:off + 64, :]
                    if tk == 0:
                        ops[h] = PO.tile([65, L], F32, tag="op", name=f"op{h}")
                        oraws[h] = SS.tile([65, L], F32, tag="oraw",
                                           name=f"oraw{h}")
                    ebs = []
                    for c0 in (0, 1024):
                        sp = PS2.tile([128, 1024], F32, tag="scr")
                        for c in (0, 512):
                            nc.tensor.matmul(
                                sp[:, c:c + 512],
                                kh[:, tk * 128:(tk + 1) * 128],
                                qh[:, c0 + c:c0 + c + 512],
                                start=True, stop=True,
                                skip_group_check=True)
                        eb = EP.tile([128, 1024], BF16, tag="eb")
                        nc.scalar.activation(eb[:], sp[:], AF.Exp,
                                             scale=0.125)
                        ebs.append(eb)
                    if prev is not None:
                        pebs, ph, ptk = prev
                        if it == 1:
                            # warm-keepers: the pipeline-fill wait on exp(0)
                            # is a PE idle long enough to re-throttle the
                            # HAM clock gate (costing ~60us of half-clock
                            # attention).  Burn it with dummy matmuls into
                            # op regions that PV(0)'s start=True clears.
                            for dm in range(8):
                                nc.tensor.matmul(
                                    ops[0][:, (dm % 4) * 512:
                                           (dm % 4) * 512 + 512],
                                    Vsb[0][:, 0:65],
                                    qT[0][:, 0:512],
                                    start=True, stop=True,
                                    skip_group_check=True)
                        for i, c0 in enumerate((0, 1024)):
                            for c in (0, 512):
                                nc.tensor.matmul(
                                    ops[ph][:, c0 + c:c0 + c + 512],
                                    Vsb[ptk][:, ph * 65:ph * 65 + 65],
                                    pebs[i][:, c:c + 512],
                                    start=(ptk == 0), stop=(ptk == TT - 1),
                                    skip_group_check=True)
                        if ptk == TT - 1:
                            normalize(ph)
                    prev = (ebs, h, tk)
                pebs, ph, ptk = prev
                for i, c0 in enumerate((0, 1024)):
                    for c in (0, 512):
                        nc.tensor.matmul(
                            ops[ph][:, c0 + c:c0 + c + 512],
                            Vsb[ptk][:, ph * 65:ph * 65 + 65],
                            pebs[i][:, c:c + 512],
                            start=False, stop=True,
                            skip_group_check=True)
                normalize(ph)

            # ---------------- Phase D: partial out-projection ------------
            # dc-outer so each wo stationary covers 4 matmuls (LDWEIGHTS
            # density matters for the HAM clock gate), drains alternate
            # ACT/DVE, output rides both DMA queues in bf16.
            with (
                tc.tile_pool(name="od", bufs=3) as OD,
                tc.tile_pool(name="pd", bufs=2, space="PSUM") as PD,
            ):
                for dc in range(DT):
                    pdt = PD.tile([128, L], F32, tag="pd")
                    if dc == 0:
                        # warm-keepers: bridge the ~8us normalize chain of
                        # the last head (DVE/GpSimd work) that would
                        # otherwise leave the PE idle and re-throttle the
                        # clock right before these 16us of matmuls.  The
                        # first real matmul's start=True clears the banks.
                        for dm in range(36):
                            nc.tensor.matmul(
                                pdt[:, (dm % 4) * 512:(dm % 4) * 512 + 512],
                                wos[0][:, 0:128],
                                qT[0][:, 0:512],
                                start=True, stop=True,
                                skip_group_check=True)
                    for e in range(ET):
                        for c in range(0, L, 512):
                            nc.tensor.matmul(
                                pdt[:, c:c + 512],
                                wos[e][:, dc * 128:(dc + 1) * 128],
                                ao[e][:, c:c + 512],
                                start=(e == 0), stop=(e == ET - 1),
                                skip_group_check=True)
                    osb = OD.tile([128, L], BF16, tag="osb")
                    if dc % 2 == 0:
                        nc.scalar.activation(osb[:], pdt[:], AF.Identity)
                    else:
                        nc.vector.tensor_copy(osb[:], pdt[:])
                    dma = nc.sync if dc % 2 == 0 else nc.scalar
                    dma.dma_start(outT[dc * 128:(dc + 1) * 128, :], osb[:])

    nc.compile()
    return nc


def _rope_tables():
    inv = 1.0 / (10000.0 ** (np.arange(0, HD, 2, dtype=np.float32) / HD))
    t = np.arange(L, dtype=np.float32)
    fr = t[:, None] * inv[None, :]                    # [L, 32]
    emb = np.concatenate([fr, fr], axis=1)            # [L, 64]
    cos, sin = np.cos(emb), np.sin(emb)               # [L, 64]
    # device layout [128, L]: row p covers head-dim i = p % 64, two heads
    # stacked per 128-partition tile; sin carries the rotate_half sign.
    i = np.arange(128) % HD
    cosb = cos.T[i, :]                                # [128, L]
    sg = np.where(i < HD // 2, -1.0, 1.0).astype(np.float32)
    sinb = sin.T[i, :] * sg[:, None]
    return np.ascontiguousarray(cosb).astype(BF16NP), \
        np.ascontiguousarray(sinb).astype(BF16NP)


def _in_maps(x, q_w, q_b, k_w, k_b, v_w, o_w):
    cosb, sinb = _rope_tables()
    qwT = np.asarray(q_w, np.float32).T.astype(BF16NP)  # [D, D] eff
    kwT = np.asarray(k_w, np.float32).T.astype(BF16NP)
    vwT = np.asarray(v_w, np.float32).T.astype(BF16NP)
    owT = np.asarray(o_w, np.float32).T.astype(BF16NP)
    xTb = [np.ascontiguousarray(x[b].T).astype(BF16NP) for b in range(B)]
    maps = []
    for c in range(NCORES):
        b, hg = divmod(c, HG)
        er = slice(hg * EL, (hg + 1) * EL)
        maps.append({
            "xT": xTb[b],
            "wq": np.ascontiguousarray(qwT[:, er]),
            "wk": np.ascontiguousarray(kwT[:, er]),
            "wv": np.ascontiguousarray(vwT[:, er]),
            "wo": np.ascontiguousarray(owT[er, :]),
            "bq": np.ascontiguousarray(
                np.asarray(q_b, np.float32)[er].reshape(ET, 128, 1)),
            "bk": np.ascontiguousarray(
                np.asarray(k_b, np.float32)[er].reshape(ET, 128, 1)),
            "cosb": cosb,
            "sinb": sinb,
            "onesd": np.ones((128, NH, 1), BF16NP),
        })
    return maps


def kernel(x, q_w, q_b, k_w, k_b, v_w, v_b, o_w, o_b):
    from concourse.bass_utils import run_bass_kernel_spmd

    x = np.asarray(x, np.float32)
    assert x.shape == (B, L, D), x.shape

    if "nc" not in _cache:
        _cache["nc"] = _build()
    nc = _cache["nc"]

    in_maps = _in_maps(x, q_w, q_b, k_w, k_b, v_w, o_w)
    res = run_bass_kernel_spmd(nc, in_maps, list(range(NCORES)))

    out = np.zeros((B, L, D), np.float32)
    for c in range(NCORES):
        b = c // HG
        out[b] += res.results[c]["outT"].T.astype(np.float32)
    # o_b, plus v_b's contribution (v_b flows through softmax-weighted
    # averaging unchanged, then through the out-projection).
    extra = np.asarray(o_b, np.float32) + \
        np.asarray(v_b, np.float32) @ np.asarray(o_w, np.float32).T
    out += extra[None, None, :]
    return out
